# revision 22
# baseline (speedup 1.0000x reference)
"""Trainium2 Bass kernel for nn_FixupBasicBlockT (dense CNN block).

Computation (reference semantics):
  y   = relu(convT2d(x + b1a, w_conv2, s=1, p=1) + b1b)           [B,256,28,28]
  m   = convT2d(y + b2a, w_conv1, s=2, p=1) * scale + b2b         [B,128,55,55]
  sel = per-sample top-128 channels of x by (max-min) range, ascending order
  id  = convT2d(x_sel + b1a, w_up, s=2, p=0)                      [B,128,56,56]
  out = relu(pad_topleft(m) + id)                                 [B,128,56,56]

Strategy: pure data parallel, 32 samples / 8 cores = 4 samples per core, no
collectives. Convolutions are expressed as per-tap matmuls accumulating in
PSUM (contraction over input channels, weights stationary, bf16 operands).
Stage A uses 1-D Winograd F(2,3) along rows: 4 transformed row-images V0..V3
(GpSimd adds) feed 4 matmul groups M0..M3 per co-half (24 matmuls instead of
36); the output combine y0 = M0+M1+M2, y1 = M1-M2-M3 is staged so only two
PSUM banks are live (M1 -> copy to SBUF -> M2 -> +/- -> M0/M3 -> relu).
The stride-2 transposed convs decompose into 4 output parity classes, each a
small set of shifted taps. Channel selection is done on-device: per-channel
f32 range -> PE transpose + ones-broadcast matmul (exact, fully on-chip) ->
pairwise-comparison rank -> one-hot permutation matrix -> a matmul gathers
the selected channels in sorted order.
"""

from contextlib import ExitStack

import numpy as np

import bass_rust
import concourse.bass as bass
import concourse.mybir as mybir
import concourse.tile as tile
from concourse.bass_utils import run_bass_kernel_spmd
from concourse.vector_clock import ScopedClock

N_CORES = 8
B, C, H = 32, 256, 28
S = B // N_CORES          # samples per core
CO1 = 128                 # inplanes (output channels)
HP = H + 2                # padded input image side (30)
YP = H + 1                # relu-output pad side (29)
NH = H * (H // 2)         # matmul N per half-image (392)
F32 = mybir.dt.float32

# consts tile column layout
IDN0, IOT0 = 0, 128       # identity[128], iota 128..255
CA0 = 256                 # cA per co-chunk [2]
BI0 = 258                 # interior bias per class [4]
BB0 = 262                 # border bias per class [4]
ONE0 = 267                # all-ones [128] block (partition broadcast matmuls)
NCONST = 395

# pb psum tile column layout (border values)
RB0 = 0                   # row borders, class b=0/1, 28 each
CB0 = 56                  # col borders, (a, h) 14 each
NPB = 112

_MAXW = 1


def _split_drain_and_barrier(self, tick_clock, wait_clock):
    """Replacement for TileContext._drain_and_barrier: this container's walrus
    rejects >1 sync wait per instruction ("Too many sync wait commands"), so
    spread the tail drain's accumulated waits over a chain of drains."""
    nc = self.nc
    drain_inst = nc.sync.drain()
    wait_clock.add_sem_waits(
        drain_inst.ins, ScopedClock({None: tick_clock.global_clock})
    )
    si = drain_inst.ins.sync_info
    if si is not None and len(si.on_wait) > _MAXW:
        waits = list(si.on_wait)
        drain_inst.ins.sync_info = bass_rust.SyncInfo(
            on_wait=waits[:_MAXW], on_update=list(si.on_update)
        )
        for ofs in range(_MAXW, len(waits), _MAXW):
            extra = nc.sync.drain()
            extra.ins.sync_info = bass_rust.SyncInfo(
                on_wait=waits[ofs : ofs + _MAXW], on_update=[]
            )
    nc.all_engine_barrier()
    assert self.sems is not None
    popped = nc._tile_sem_poison_stack.pop()
    assert popped is self._sem_poison
    nc.clear_and_free_semaphores(list(self.sems.allocated().values()))
    nc.all_engine_barrier()


tile.TileContext._drain_and_barrier = _split_drain_and_barrier

_NOPN = [0]


def _split_multi_waits(nc):
    """Walrus here accepts at most one sync wait per instruction; hoist extra
    waits onto same-engine carrier NOPs placed immediately before."""
    for fn in nc.m.functions:
        for blk in fn.blocks:
            out = []
            for ins in blk.instructions:
                si = ins.sync_info
                if si is not None and len(si.on_wait) > 1:
                    waits = list(si.on_wait)
                    for w in waits[:-1]:
                        _NOPN[0] += 1
                        out.append(mybir.InstNoOp(
                            name=f"wsplit_{_NOPN[0]}",
                            engine=ins.engine,
                            sync_info=mybir.SyncInfo(on_wait=[w], on_update=[]),
                            bass_nofuse=True,
                        ))
                    ins.sync_info = mybir.SyncInfo(
                        on_wait=[waits[-1]], on_update=list(si.on_update)
                    )
                out.append(ins)
            blk.instructions = out


def build_nc(mm_dt=mybir.dt.bfloat16):
    """Build the per-core Bass kernel (same program on all 8 cores)."""
    nc = bass.Bass()
    adt = mm_dt  # dtype for everything the big matmuls consume
    xpb_d = nc.declare_dram_parameter("xpb", [S, 2, 128, HP * HP], adt, isOutput=False)
    xv_d = nc.declare_dram_parameter("xv", [S, 2, 4, 128, 14 * HP], adt, isOutput=False)
    xpf_d = nc.declare_dram_parameter("xpf", [S, 2, 128, H * H], F32, isOutput=False)
    w2_d = nc.declare_dram_parameter("w2", [2, 4, 128, 3 * 2 * 128], adt, isOutput=False)
    w1_d = nc.declare_dram_parameter("w1", [2, 128, 9 * 128], adt, isOutput=False)
    wu_d = nc.declare_dram_parameter("wu", [128, 4 * 128], adt, isOutput=False)
    cs_d = nc.declare_dram_parameter("cs", [128, NCONST], F32, isOutput=False)
    out_d = nc.declare_dram_parameter("out", [S, 128, 56 * 56], F32, isOutput=True)

    with tile.TileContext(nc) as tc, ExitStack() as ctx:
        wpool = ctx.enter_context(tc.tile_pool(name="wpool", bufs=1))
        work = ctx.enter_context(tc.tile_pool(name="work", bufs=2))
        psW = ctx.enter_context(tc.tile_pool(name="psW", bufs=2, space="PSUM"))
        psmm = ctx.enter_context(tc.tile_pool(name="psmm", bufs=3, space="PSUM"))
        psbp = ctx.enter_context(tc.tile_pool(name="psbp", bufs=2, space="PSUM"))
        psbc = ctx.enter_context(tc.tile_pool(name="psbc", bufs=1, space="PSUM"))

        # --- resident weights/constants (cs + xpb(0) + w2 first: stage A of
        # sample 0 can start as soon as they land) ---
        w2_t = [wpool.tile([128, 4 * 3 * 2 * 128], adt, tag=f"w2_{c}", name=f"w2_{c}") for c in range(2)]
        w1_t = [wpool.tile([128, 9 * 128], adt, tag=f"w1_{c}", name=f"w1_{c}") for c in range(2)]
        wu_t = wpool.tile([128, 4 * 128], adt, tag="wu", name="wu")
        cs_t = wpool.tile([128, NCONST], F32, tag="cs", name="cs")

        ident = cs_t[:, IDN0 : IDN0 + 128]
        iota = cs_t[:, IOT0 : IOT0 + 128]

        # warm the Scalar engine's activation table during the DMA head so the
        # first real RELU doesn't pay the ~1.3us ACT_TABLE_LOAD. Bias is an AP
        # (a float bias would create a const tensor whose preamble DMA+barrier
        # delays the bulk input loads by ~3us).
        warm = work.tile([128, 2], F32, tag="warm", name="warm", bufs=1)
        nc.vector.memset(warm[:], 0.0)
        nc.scalar.activation(
            warm[:, 0:1], warm[:, 1:2], mybir.ActivationFunctionType.Relu,
            bias=warm[:, 1:2])

        def w2s(ci2, xi, kw, co2):  # stage-A lhsT Winograd slice U[xi, kw]
            o = xi * 768 + (kw * 2 + co2) * 128
            return w2_t[ci2][:, o : o + 128]

        def w1s(ci2, t9):  # stage-B lhsT tap slice
            return w1_t[ci2][:, t9 * 128 : (t9 + 1) * 128]

        def wus(k):  # up-conv lhsT class slice
            return wu_t[:, k * 128 : (k + 1) * 128]

        state = {}  # per-sample tiles shared between emit stages

        def emit_xpb_load(s, c=None):
            if s not in state:
                state[s] = {}
            st = state[s]
            if "xb" not in st:
                st["xb"] = [work.tile([128, HP * HP], adt, tag=f"xb{cc}",
                                      name=f"xb{cc}", bufs=4) for cc in range(2)]
            cs_ = range(2) if c is None else [c]
            for cc in cs_:
                nc.sync.dma_start(st["xb"][cc][:], xpb_d[s, cc])

        def emit_xv_load(s, xi=None):
            if s not in state:
                state[s] = {}
            st = state[s]
            if "xv" not in st:
                st["xv"] = [work.tile([128, 4 * 14 * HP], adt, tag=f"v{cc}",
                                      name=f"v{cc}", bufs=3) for cc in range(2)]
            xis = range(4) if xi is None else [xi]
            for cc in range(2):
                for x_ in xis:
                    nc.sync.dma_start(
                        st["xv"][cc][:, x_ * 420 : (x_ + 1) * 420],
                        xv_d[s, cc, x_])

        def emit_xpf_load(s):
            st = state[s]
            st["xf"] = [work.tile([128, H * H], F32, tag=f"xf{cc}",
                                  name=f"xf{cc}", bufs=4) for cc in range(2)]
            for cc in range(2):
                nc.sync.dma_start(st["xf"][cc][:], xpf_d[s, cc])

        def emit_front(s):
            """Stage A via 1-D Winograd F(2,3) on rows. GpSimd builds the four
            transformed row-images V0..V3 from the padded bf16 input; the PE
            runs 4 accumulation groups M0..M3 per co-half (6 matmuls each, 24
            total vs 36 direct); Vector combines y0 = M0+M1+M2 and
            y1 = M1-M2-M3 with at most one PSUM operand per op (two live
            banks: M1 -> Scalar copy to SBUF -> M2 -> +- -> M0/M3)."""
            st = state[s]
            x3 = [t[:].rearrange("p (h w) -> p h w", w=HP) for t in st["xb"]]
            # V transform rows are precomputed on the host and DMA'd in: no
            # engine ever sits on the front's critical path.
            V4 = [t[:].rearrange("p (x i w) -> p x i w", x=4, w=HP)
                  for t in st["xv"]]

            y_t = [work.tile([128, YP * YP], adt, tag=f"y{c}", name=f"y{c}", bufs=3)
                   for c in range(2)]
            y3 = [t[:].rearrange("p (h w) -> p h w", w=YP) for t in y_t]
            for c in range(2):
                nc.gpsimd.memset(y_t[c][:, 0:YP], 0.0)          # row 0
                nc.gpsimd.memset(y3[c][:, 1:YP, 0:1], 0.0)      # col 0

            def mgroup(xi, co2):
                pm = psW.tile([128, NH], F32, tag="w", name="w")
                n = 0
                for c in range(2):
                    for kw in range(3):
                        nc.tensor.matmul(
                            pm[:], w2s(c, xi, kw, co2),
                            V4[c][:, xi, :, kw : kw + H],
                            start=(n == 0), stop=(n == 5))
                        n += 1
                return pm

            # co2-interleaved group order (xi = 1,1,2,2,0,0,3,3): the first
            # two groups need only the xi=1 weight chunk (DMA'd first), and
            # each combine's PSUM operand is ready well before Vector gets to
            # it. psW rotates a,b,a,b: each realloc waits only the staged
            # reads (Scalar copy / Vector u,v) of the bank two allocs back.
            s1 = {}
            u = {}
            v_ = {}
            bias = [cs_t[:, CA0 + c2 : CA0 + c2 + 1] for c2 in range(2)]
            pM1 = {c2: mgroup(1, c2) for c2 in range(2)}
            for c2 in range(2):
                s1[c2] = work.tile([128, NH], F32, tag=f"ws1{c2}", name="ws1")
                nc.scalar.activation(
                    s1[c2][:], pM1[c2][:], mybir.ActivationFunctionType.Copy)
            pM2 = {c2: mgroup(2, c2) for c2 in range(2)}
            for c2 in range(2):
                u[c2] = work.tile([128, NH], F32, tag=f"wu{c2}", name="wu_")
                v_[c2] = work.tile([128, NH], F32, tag=f"wv{c2}", name="wv_")
                nc.vector.tensor_sub(u[c2][:], s1[c2][:], pM2[c2][:])
                nc.vector.tensor_add(v_[c2][:], s1[c2][:], pM2[c2][:])
            pM0 = {c2: mgroup(0, c2) for c2 in range(2)}
            for c2 in range(2):
                t0 = work.tile([128, NH], F32, tag=f"wt0{c2}", name="wt0")
                nc.vector.tensor_add(t0[:], v_[c2][:], pM0[c2][:])
                nc.scalar.activation(
                    y3[c2][:, 1:29:2, 1 : 1 + H],
                    t0[:].rearrange("p (i j) -> p i j", j=H),
                    mybir.ActivationFunctionType.Relu, bias=bias[c2], scale=1.0)
            pM3 = {c2: mgroup(3, c2) for c2 in range(2)}
            for c2 in range(2):
                t1 = work.tile([128, NH], F32, tag=f"wt1{c2}", name="wt1")
                nc.vector.tensor_sub(t1[:], u[c2][:], pM3[c2][:])
                nc.scalar.activation(
                    y3[c2][:, 2:29:2, 1 : 1 + H],
                    t1[:].rearrange("p (i j) -> p i j", j=H),
                    mybir.ActivationFunctionType.Relu, bias=bias[c2], scale=1.0)
            st["x3"] = x3
            st["y3"] = y3

        def emit_rng_stats(s):
            """Vector-side selection stats (exact f32 max/min)."""
            st = state[s]
            xf = st["xf"]
            mx = work.tile([128, 2], F32, tag="mx", name="mx", bufs=4)
            mn = work.tile([128, 2], F32, tag="mn", name="mn", bufs=4)
            rng_t = work.tile([128, 2], F32, tag="rng", name="rng", bufs=4)
            for c in range(2):
                nc.vector.tensor_reduce(
                    mx[:, c : c + 1], xf[c][:],
                    axis=mybir.AxisListType.X, op=mybir.AluOpType.max)
                nc.vector.tensor_reduce(
                    mn[:, c : c + 1], xf[c][:],
                    axis=mybir.AxisListType.X, op=mybir.AluOpType.min)
            nc.vector.tensor_sub(rng_t[:], mx[:], mn[:])
            st["rng"] = rng_t

        def emit_rng_selA(s):
            """PE transposes of the range columns (placed right after
            front(s)'s matmuls in the PE stream)."""
            st = state[s]
            rng_t = st["rng"]
            pbc = psbc.tile([128, 512], F32, tag="bc", name="bc")
            rngT = work.tile([128, 256], F32, tag="rngT", name="rngT", bufs=2)
            for h in range(2):
                nc.tensor.transpose(
                    pbc[0:1, 256 + 128 * h : 384 + 128 * h],
                    rng_t[:, h : h + 1], ident)
                nc.vector.tensor_copy(
                    rngT[0:1, 128 * h : 128 * (h + 1)],
                    pbc[0:1, 256 + 128 * h : 384 + 128 * h])
            st["pbc"], st["rngT"] = pbc, rngT

        def emit_rng_selB(s):
            """Broadcast matmul (exact: single 1.0*v products) + rank +
            one-hot permutation build."""
            st = state[s]
            pbc, rngT, rng_t = st.pop("pbc"), st.pop("rngT"), st.pop("rng")
            nc.tensor.matmul(
                pbc[:, 0:256], cs_t[0:1, ONE0 : ONE0 + 128],
                rngT[0:1, 0:256], start=True, stop=True)
            rank_t = work.tile([128, 2], F32, tag="rank", name="rank", bufs=4)
            P_t = work.tile([128, 256], adt, tag="pt", name="pt", bufs=4)
            for c in range(2):
                mk = work.tile([128, 256], F32, tag=f"mask{c}", name=f"mask{c}", bufs=4)
                nc.vector.tensor_scalar(
                    mk[:], pbc[:, 0:256], rng_t[:, c : c + 1], None,
                    mybir.AluOpType.is_lt)
                nc.vector.tensor_reduce(
                    rank_t[:, c : c + 1], mk[:],
                    axis=mybir.AxisListType.X, op=mybir.AluOpType.add)
                nc.vector.tensor_scalar(
                    P_t[:, c * 128 : (c + 1) * 128], iota, rank_t[:, c : c + 1],
                    None, mybir.AluOpType.is_equal)
            st["P_t"] = P_t

        def emit_mid_mm(s):
            """x_sel gather + border-value matmuls (PE work, depth-1 ahead)."""
            st = state[s]
            x3, P_t = st["x3"], st["P_t"]
            # gather selected channels via one-hot matmul
            x_sel = work.tile([128, H * H], adt, tag="xsel", name="xsel")
            for h in range(2):
                ps = psmm.tile([128, NH], F32, tag="mm", name="mm")
                for c in range(2):
                    nc.tensor.matmul(
                        ps[:], P_t[:, c * 128 : (c + 1) * 128],
                        x3[c][:, 1 + 14 * h : 15 + 14 * h, 1 : 1 + H],
                        start=(c == 0), stop=(c == 1))
                nc.scalar.activation(
                    x_sel[:, NH * h : NH * (h + 1)], ps[:],
                    mybir.ActivationFunctionType.Copy)
            s3 = x_sel[:].rearrange("p (i j) -> p i j", j=H)

            # border-value matmuls (identity-only contributions)
            pb = psbp.tile([128, NPB], F32, tag="pb", name="pb")
            for b in range(2):      # row border, classes (0,b), output row P=0
                nc.tensor.matmul(
                    pb[:, RB0 + b * 28 : RB0 + (b + 1) * 28],
                    wus(b), s3[:, 0, :], start=True, stop=True)
            for a in range(2):      # col border, classes (a,0), output col Q=0
                for h in range(2):
                    o = CB0 + (a * 2 + h) * 14
                    nc.tensor.matmul(
                        pb[:, o : o + 14], wus(a * 2),
                        s3[:, 14 * h : 14 * h + 14, 0],
                        start=True, stop=True)
            st["s3"], st["pb"] = s3, pb

        def emit_back(s):
            """Stage B/C: stride-2 convT + upsample branch, fused relu, store.
            h-outer so rows 0-27 can stream out while rows 28-55 compute."""
            st = state.pop(s)
            y3, s3, pb = st["y3"], st["s3"], st["pb"]
            G_t = work.tile([128, 56 * 56], F32, tag="g", name="g")
            g5 = G_t[:].rearrange("p (i x j y) -> p i x j y", x=2, y=2, j=H)
            rowtaps = {0: [(0, 1), (2, 0)], 1: [(1, 1)]}
            coltaps = rowtaps
            for h in range(2):
                for a in range(2):
                    for b in range(2):
                        k = a * 2 + b
                        n_taps = 2 * len(rowtaps[a]) * len(coltaps[b])
                        psC = psmm.tile([128, NH], F32, tag="mm", name="mm")
                        p3 = psC[:].rearrange("p (i j) -> p i j", j=H)
                        nc.tensor.matmul(
                            psC[:], wus(k), s3[:, 14 * h : 14 * h + 14, :],
                            start=True, stop=False)
                        n_mm = 0
                        for kh, ro in rowtaps[a]:
                            for kw, co_ in coltaps[b]:
                                for c in range(2):
                                    n_mm += 1
                                    nc.tensor.matmul(
                                        psC[:], w1s(c, kh * 3 + kw),
                                        y3[c][:, ro + 14 * h : ro + 14 * h + 14,
                                                 co_ : co_ + H],
                                        start=False, stop=(n_mm == n_taps))
                        i0 = 1 if (a == 0 and h == 0) else 0
                        j0 = 1 if b == 0 else 0
                        nc.scalar.activation(
                            g5[:, 14 * h + i0 : 14 * h + 14, a, j0:H, b],
                            p3[:, i0:14, j0:H],
                            mybir.ActivationFunctionType.Relu,
                            bias=cs_t[:, BI0 + k : BI0 + k + 1], scale=1.0)
                        # border fixes: output positions whose main-branch
                        # input is pure padding get relu(identity + border
                        # bias) instead
                        if a == 0 and h == 0:
                            nc.scalar.activation(
                                g5[:, 0, 0, :, b],
                                pb[:, RB0 + b * 28 : RB0 + b * 28 + 28],
                                mybir.ActivationFunctionType.Relu,
                                bias=cs_t[:, BB0 + k : BB0 + k + 1], scale=1.0)
                        if b == 0:
                            i0 = 1 if (a == 0 and h == 0) else 0
                            o = CB0 + (a * 2 + h) * 14
                            nc.scalar.activation(
                                g5[:, 14 * h + i0 : 14 * h + 14, a, 0, 0],
                                pb[:, o + i0 : o + 14],
                                mybir.ActivationFunctionType.Relu,
                                bias=cs_t[:, BB0 + k : BB0 + k + 1], scale=1.0)
                nc.sync.dma_start(
                    out_d[s][:, h * 1568 : (h + 1) * 1568],
                    G_t[:, h * 1568 : (h + 1) * 1568])

        # --- DMA emission order == arrival order: the head is exactly what
        # the first front group needs (xpb0 c0 + w2 c0), weights not needed
        # until the first mid/back phases go after all sample loads they
        # would otherwise delay. ---
        emit_xv_load(0, xi=1)
        for c in range(2):
            nc.sync.dma_start(w2_t[c][:, 768 : 2 * 768], w2_d[c, 1])
        emit_xv_load(0, xi=2)
        for c in range(2):
            nc.sync.dma_start(w2_t[c][:, 2 * 768 : 3 * 768], w2_d[c, 2])
        nc.sync.dma_start(cs_t[:], cs_d[:])
        emit_xv_load(0, xi=0)
        for c in range(2):
            nc.sync.dma_start(w2_t[c][:, 0:768], w2_d[c, 0])
        emit_xv_load(0, xi=3)
        for c in range(2):
            nc.sync.dma_start(w2_t[c][:, 3 * 768 : 4 * 768], w2_d[c, 3])
        emit_xpf_load(0)
        emit_xv_load(1)
        emit_xpb_load(0)
        emit_xpf_load(1)
        emit_xv_load(2)
        nc.sync.dma_start(wu_t[:], wu_d[:])
        emit_xpb_load(1)
        emit_xv_load(3)
        emit_xpf_load(2)
        emit_xpf_load(3)
        for c in range(2):
            nc.sync.dma_start(w1_t[c][:], w1_d[c])
        emit_xpb_load(2)
        emit_xpb_load(3)

        # software pipeline (PE stream): F0, F1, tr0, bc0, M0 | tr1, bc1,
        # F2, M1, B0 | tr2, bc2, F3, M2, B1 | tr3, bc3, M3, B2 | B3. The
        # selection chain for sample s+1 leads each iteration: its broadcast
        # rides between front phases and its rank chain (Vector) finishes
        # while the PE is inside front(s+2), so mid(s+1) is never gated.
        emit_front(0)
        emit_rng_stats(0)
        emit_front(1)
        emit_rng_selA(0)
        emit_rng_selB(0)
        emit_rng_stats(1)
        emit_mid_mm(0)
        for s in range(S):
            if s + 1 <= S - 2:
                emit_rng_selA(s + 1)
                emit_rng_selB(s + 1)
            if s + 2 < S:
                emit_front(s + 2)
                emit_rng_stats(s + 2)
            if s + 1 < S:
                emit_mid_mm(s + 1)
            if s == S - 3:
                # last sample's selection chain rides inside this iteration so
                # its rank chain is hidden under back(s) instead of exposed at
                # the pipeline tail.
                emit_rng_selA(S - 1)
                emit_rng_selB(S - 1)
            emit_back(s)
    _split_multi_waits(nc)
    return nc


def prep_inputs(x, w_conv2, w_conv1, w_up, bias1a, bias1b, bias2a, bias2b, scale,
                w_np_dt=np.float32):
    """Host-side input transforms shared by all cores (weights/constants)."""
    f = np.float32
    b1a, b1b = f(bias1a[0]), f(bias1b[0])
    b2a, b2b, sc = f(bias2a[0]), f(bias2b[0]), f(scale[0])

    xp = np.zeros((B, 2, 128, HP, HP), dtype=np.float32)
    xp[:, :, :, 1 : 1 + H, 1 : 1 + H] = x.reshape(B, 2, 128, H, H)
    xpb = xp.reshape(B, 2, 128, HP * HP).astype(w_np_dt)
    xpf = np.ascontiguousarray(x.reshape(B, 2, 128, H * H), dtype=np.float32)
    # host-side 1-D Winograd F(2,3) input transform (rows): V0=d0-d2,
    # V1=d1+d2, V2=d2-d1, V3=d1-d3 with d_k = padded rows 2i+k.
    dk = [xp[:, :, :, k : k + 27 : 2, :] for k in range(4)]
    xv = np.stack([dk[0] - dk[2], dk[1] + dk[2], dk[2] - dk[1],
                   dk[1] - dk[3]], axis=3)               # [B,2,128,4,14,30]
    xv = np.ascontiguousarray(
        xv.transpose(0, 1, 3, 2, 4, 5)
    ).reshape(B, 2, 4, 128, 14 * HP).astype(w_np_dt)

    wf = w_conv2[:, :, ::-1, ::-1]  # flip -> correlation form
    # 1-D Winograd F(2,3) weight transform along kh (correlation taps g0..g2):
    # U0 = g0, U1 = (g0+g1+g2)/2, U2 = (g0-g1+g2)/2, U3 = g2.
    U = np.stack([
        wf[:, :, 0, :],
        0.5 * (wf[:, :, 0, :] + wf[:, :, 1, :] + wf[:, :, 2, :]),
        0.5 * (wf[:, :, 0, :] - wf[:, :, 1, :] + wf[:, :, 2, :]),
        wf[:, :, 2, :],
    ])                                                  # [4, 256ci, 256co, 3kw]
    w2 = np.ascontiguousarray(
        U.reshape(4, 2, 128, 2, 128, 3).transpose(1, 0, 2, 5, 3, 4)
    ).reshape(2, 4, 128, 3 * 2 * 128)
    w1s = (w_conv1 * sc).astype(np.float32)
    w1 = np.ascontiguousarray(
        w1s.reshape(2, 128, 128, 3, 3).transpose(0, 1, 3, 4, 2)
    ).reshape(2, 128, 9 * 128)
    wu = np.ascontiguousarray(w_up.transpose(0, 2, 3, 1)).reshape(128, 4 * 128)

    cs = np.zeros((128, NCONST), dtype=np.float32)
    cs[:, IDN0 : IDN0 + 128] = np.eye(128, dtype=np.float32)
    cs[:, IOT0 : IOT0 + 128] = np.arange(128, 256, dtype=np.float32)[None, :]
    cA = b1a * w_conv2.sum(axis=(0, 2, 3)) + b1b               # [256]
    cs[:, CA0 : CA0 + 2] = cA.reshape(2, 128).T
    cI = b1a * w_up.sum(axis=0)                                # [128,2,2]
    # fold bias2a into the per-class interior bias: conv(y + b2a) adds
    # b2a * (sum of the class's taps over all input channels) per output
    # channel (exact for interior positions; b2a is 0 under Fixup init).
    wsum = w1s.sum(axis=0)                                     # [128,3,3]
    ktaps = {0: [0, 2], 1: [1]}
    for a in range(2):
        for b_ in range(2):
            Sk = wsum[:, ktaps[a], :][:, :, ktaps[b_]].sum(axis=(1, 2))
            cs[:, BI0 + a * 2 + b_] = cI[:, a, b_] + b2b + b2a * Sk
            cs[:, BB0 + a * 2 + b_] = cI[:, a, b_]
    cs[:, ONE0 : ONE0 + 128] = 1.0

    shared = {"w2": w2.astype(w_np_dt), "w1": w1.astype(w_np_dt),
              "wu": wu.astype(w_np_dt), "cs": cs}
    in_maps = []
    for i in range(N_CORES):
        m = dict(shared)
        m["xpb"] = np.ascontiguousarray(xpb[i * S : (i + 1) * S])
        m["xv"] = np.ascontiguousarray(xv[i * S : (i + 1) * S])
        m["xpf"] = np.ascontiguousarray(xpf[i * S : (i + 1) * S])
        in_maps.append(m)
    return in_maps


_NC_CACHE = {}


def _get_nc(mm_dt):
    key = str(mm_dt)
    if key not in _NC_CACHE:
        _NC_CACHE[key] = build_nc(mm_dt)
    return _NC_CACHE[key]


MM_DT = mybir.dt.bfloat16


def run(inputs, mm_dt=None, trace=False):
    mm_dt = MM_DT if mm_dt is None else mm_dt
    nc = _get_nc(mm_dt)
    in_maps = prep_inputs(**inputs, w_np_dt=mybir.dt.np(mm_dt))
    res = run_bass_kernel_spmd(nc, in_maps, list(range(N_CORES)), trace=trace)
    out = np.empty((B, CO1, 56, 56), dtype=np.float32)
    for i in range(N_CORES):
        out[i * S : (i + 1) * S] = res.results[i]["out"].reshape(S, CO1, 56, 56)
    return out, res


def kernel(**inputs):
    out, _ = run(inputs)
    return out


# revision 24
# speedup vs baseline: 1.0372x; 1.0372x over previous
"""Trainium2 Bass kernel for nn_FixupBasicBlockT (dense CNN block).

Computation (reference semantics):
  y   = relu(convT2d(x + b1a, w_conv2, s=1, p=1) + b1b)           [B,256,28,28]
  m   = convT2d(y + b2a, w_conv1, s=2, p=1) * scale + b2b         [B,128,55,55]
  sel = per-sample top-128 channels of x by (max-min) range, ascending order
  id  = convT2d(x_sel + b1a, w_up, s=2, p=0)                      [B,128,56,56]
  out = relu(pad_topleft(m) + id)                                 [B,128,56,56]

Strategy: pure data parallel, 32 samples / 8 cores = 4 samples per core, no
collectives. Convolutions are expressed as per-tap matmuls accumulating in
PSUM (contraction over input channels, weights stationary, bf16 operands).
Stage A uses 1-D Winograd F(2,3) along rows: 4 transformed row-images V0..V3
(GpSimd adds) feed 4 matmul groups M0..M3 per co-half (24 matmuls instead of
36); the output combine y0 = M0+M1+M2, y1 = M1-M2-M3 is staged so only two
PSUM banks are live (M1 -> copy to SBUF -> M2 -> +/- -> M0/M3 -> relu).
The stride-2 transposed convs decompose into 4 output parity classes, each a
small set of shifted taps. Channel selection is done on-device: per-channel
f32 range -> PE transpose + ones-broadcast matmul (exact, fully on-chip) ->
pairwise-comparison rank -> one-hot permutation matrix -> a matmul gathers
the selected channels in sorted order.
"""

from contextlib import ExitStack

import numpy as np

import bass_rust
import concourse.bass as bass
import concourse.mybir as mybir
import concourse.tile as tile
from concourse.bass_utils import run_bass_kernel_spmd
from concourse.vector_clock import ScopedClock

N_CORES = 8
B, C, H = 32, 256, 28
S = B // N_CORES          # samples per core
CO1 = 128                 # inplanes (output channels)
HP = H + 2                # padded input image side (30)
YP = H + 1                # relu-output pad side (29)
NH = H * (H // 2)         # matmul N per half-image (392)
F32 = mybir.dt.float32

# consts tile column layout
IDN0, IOT0 = 0, 128       # identity[128], iota 128..255
CA0 = 256                 # cA per co-chunk [2]
BI0 = 258                 # interior bias per class [4]
BB0 = 262                 # border bias per class [4]
ONE0 = 267                # all-ones [128] block (partition broadcast matmuls)
NCONST = 395

# pb psum tile column layout (border values)
RB0 = 0                   # row borders, class b=0/1, 28 each
CB0 = 56                  # col borders, (a, h) 14 each
NPB = 112

_MAXW = 1


def _split_drain_and_barrier(self, tick_clock, wait_clock):
    """Replacement for TileContext._drain_and_barrier: this container's walrus
    rejects >1 sync wait per instruction ("Too many sync wait commands"), so
    spread the tail drain's accumulated waits over a chain of drains."""
    nc = self.nc
    drain_inst = nc.sync.drain()
    wait_clock.add_sem_waits(
        drain_inst.ins, ScopedClock({None: tick_clock.global_clock})
    )
    si = drain_inst.ins.sync_info
    if si is not None and len(si.on_wait) > _MAXW:
        waits = list(si.on_wait)
        drain_inst.ins.sync_info = bass_rust.SyncInfo(
            on_wait=waits[:_MAXW], on_update=list(si.on_update)
        )
        for ofs in range(_MAXW, len(waits), _MAXW):
            extra = nc.sync.drain()
            extra.ins.sync_info = bass_rust.SyncInfo(
                on_wait=waits[ofs : ofs + _MAXW], on_update=[]
            )
    nc.all_engine_barrier()
    assert self.sems is not None
    popped = nc._tile_sem_poison_stack.pop()
    assert popped is self._sem_poison
    nc.clear_and_free_semaphores(list(self.sems.allocated().values()))
    nc.all_engine_barrier()


tile.TileContext._drain_and_barrier = _split_drain_and_barrier

_NOPN = [0]


def _split_multi_waits(nc):
    """Walrus here accepts at most one sync wait per instruction; hoist extra
    waits onto same-engine carrier NOPs placed immediately before."""
    for fn in nc.m.functions:
        for blk in fn.blocks:
            out = []
            for ins in blk.instructions:
                si = ins.sync_info
                if si is not None and len(si.on_wait) > 1:
                    waits = list(si.on_wait)
                    for w in waits[:-1]:
                        _NOPN[0] += 1
                        out.append(mybir.InstNoOp(
                            name=f"wsplit_{_NOPN[0]}",
                            engine=ins.engine,
                            sync_info=mybir.SyncInfo(on_wait=[w], on_update=[]),
                            bass_nofuse=True,
                        ))
                    ins.sync_info = mybir.SyncInfo(
                        on_wait=[waits[-1]], on_update=list(si.on_update)
                    )
                out.append(ins)
            blk.instructions = out


def build_nc(mm_dt=mybir.dt.bfloat16):
    """Build the per-core Bass kernel (same program on all 8 cores)."""
    nc = bass.Bass()
    adt = mm_dt  # dtype for everything the big matmuls consume
    xpb_d = nc.declare_dram_parameter("xpb", [S, 2, 128, HP * HP], adt, isOutput=False)
    xpf_d = nc.declare_dram_parameter("xpf", [S, 2, 128, H * H], F32, isOutput=False)
    w2_d = nc.declare_dram_parameter("w2", [2, 4, 128, 3 * 2 * 128], adt, isOutput=False)
    w1_d = nc.declare_dram_parameter("w1", [2, 128, 9 * 128], adt, isOutput=False)
    wu_d = nc.declare_dram_parameter("wu", [128, 4 * 128], adt, isOutput=False)
    cs_d = nc.declare_dram_parameter("cs", [128, NCONST], F32, isOutput=False)
    out_d = nc.declare_dram_parameter("out", [S, 128, 56 * 56], F32, isOutput=True)

    with tile.TileContext(nc) as tc, ExitStack() as ctx:
        wpool = ctx.enter_context(tc.tile_pool(name="wpool", bufs=1))
        work = ctx.enter_context(tc.tile_pool(name="work", bufs=2))
        psW = ctx.enter_context(tc.tile_pool(name="psW", bufs=2, space="PSUM"))
        psmm = ctx.enter_context(tc.tile_pool(name="psmm", bufs=3, space="PSUM"))
        psbp = ctx.enter_context(tc.tile_pool(name="psbp", bufs=2, space="PSUM"))
        psbc = ctx.enter_context(tc.tile_pool(name="psbc", bufs=1, space="PSUM"))

        # --- resident weights/constants (cs + xpb(0) + w2 first: stage A of
        # sample 0 can start as soon as they land) ---
        w2_t = [wpool.tile([128, 4 * 3 * 2 * 128], adt, tag=f"w2_{c}", name=f"w2_{c}") for c in range(2)]
        w1_t = [wpool.tile([128, 9 * 128], adt, tag=f"w1_{c}", name=f"w1_{c}") for c in range(2)]
        wu_t = wpool.tile([128, 4 * 128], adt, tag="wu", name="wu")
        cs_t = wpool.tile([128, NCONST], F32, tag="cs", name="cs")

        ident = cs_t[:, IDN0 : IDN0 + 128]
        iota = cs_t[:, IOT0 : IOT0 + 128]

        # warm the Scalar engine's activation table during the DMA head so the
        # first real RELU doesn't pay the ~1.3us ACT_TABLE_LOAD. Bias is an AP
        # (a float bias would create a const tensor whose preamble DMA+barrier
        # delays the bulk input loads by ~3us).
        warm = work.tile([128, 2], F32, tag="warm", name="warm", bufs=1)
        nc.vector.memset(warm[:], 0.0)
        nc.scalar.activation(
            warm[:, 0:1], warm[:, 1:2], mybir.ActivationFunctionType.Relu,
            bias=warm[:, 1:2])

        def w2s(ci2, xi, kw, co2):  # stage-A lhsT Winograd slice U[xi, kw]
            o = xi * 768 + (kw * 2 + co2) * 128
            return w2_t[ci2][:, o : o + 128]

        def w1s(ci2, t9):  # stage-B lhsT tap slice
            return w1_t[ci2][:, t9 * 128 : (t9 + 1) * 128]

        def wus(k):  # up-conv lhsT class slice
            return wu_t[:, k * 128 : (k + 1) * 128]

        state = {}  # per-sample tiles shared between emit stages

        def emit_xpb_load(s, c=None):
            if s not in state:
                state[s] = {}
            st = state[s]
            if "xb" not in st:
                st["xb"] = [work.tile([128, HP * HP], adt, tag=f"xb{cc}",
                                      name=f"xb{cc}", bufs=4) for cc in range(2)]
            cs_ = range(2) if c is None else [c]
            for cc in cs_:
                nc.sync.dma_start(st["xb"][cc][:], xpb_d[s, cc])

        def emit_xpf_load(s):
            st = state[s]
            st["xf"] = [work.tile([128, H * H], F32, tag=f"xf{cc}",
                                  name=f"xf{cc}", bufs=4) for cc in range(2)]
            for cc in range(2):
                nc.sync.dma_start(st["xf"][cc][:], xpf_d[s, cc])

        def emit_front(s, c1_eng=None):
            """Stage A via 1-D Winograd F(2,3) on rows. GpSimd builds the four
            transformed row-images V0..V3 from the padded bf16 input; the PE
            runs 4 accumulation groups M0..M3 per co-half (6 matmuls each, 24
            total vs 36 direct); Vector combines y0 = M0+M1+M2 and
            y1 = M1-M2-M3 with at most one PSUM operand per op (two live
            banks: M1 -> Scalar copy to SBUF -> M2 -> +- -> M0/M3)."""
            st = state[s]
            x3 = [t[:].rearrange("p (h w) -> p h w", w=HP) for t in st["xb"]]

            # V transforms (GpSimd, SBUF only): V0=d0-d2 V1=d1+d2 V2=d2-d1
            # V3=d1-d3 where d_k = input rows 2i+k, i=0..13. Emitted in the
            # matmul-group order (xi = 1,2,0,3) and ci-paired so the PE's
            # first group is never gated on a late transform.
            V4 = []
            for c in range(2):
                v_t = work.tile([128, 4 * 14 * HP], adt, tag=f"v{c}",
                                name=f"v{c}", bufs=3)
                V4.append(v_t[:].rearrange("p (x i w) -> p x i w", x=4, w=HP))
            d = lambda c, k: x3[c][:, k : k + 27 : 2, :]
            c1_e = nc.vector if c1_eng is None else c1_eng
            for xi, op, ka, kb in ((1, "add", 1, 2), (2, "sub", 2, 1),
                                   (0, "sub", 0, 2), (3, "sub", 1, 3)):
                # split across GpSimd (c0) and Vector (c1): halves the
                # per-sample transform latency so the pipeline fill is never
                # gated on a single engine's serial transform chain.
                getattr(nc.gpsimd, f"tensor_{op}")(
                    V4[0][:, xi], d(0, ka), d(0, kb))
                getattr(c1_e, f"tensor_{op}")(
                    V4[1][:, xi], d(1, ka), d(1, kb))

            y_t = [work.tile([128, YP * YP], adt, tag=f"y{c}", name=f"y{c}", bufs=3)
                   for c in range(2)]
            y3 = [t[:].rearrange("p (h w) -> p h w", w=YP) for t in y_t]
            for c in range(2):
                nc.gpsimd.memset(y_t[c][:, 0:YP], 0.0)          # row 0
                nc.gpsimd.memset(y3[c][:, 1:YP, 0:1], 0.0)      # col 0

            def mgroup(xi, co2):
                pm = psW.tile([128, NH], F32, tag="w", name="w")
                n = 0
                for c in range(2):
                    for kw in range(3):
                        nc.tensor.matmul(
                            pm[:], w2s(c, xi, kw, co2),
                            V4[c][:, xi, :, kw : kw + H],
                            start=(n == 0), stop=(n == 5))
                        n += 1
                return pm

            # co2-interleaved group order (xi = 1,1,2,2,0,0,3,3): the first
            # two groups need only the xi=1 weight chunk (DMA'd first), and
            # each combine's PSUM operand is ready well before Vector gets to
            # it. psW rotates a,b,a,b: each realloc waits only the staged
            # reads (Scalar copy / Vector u,v) of the bank two allocs back.
            s1 = {}
            u = {}
            v_ = {}
            bias = [cs_t[:, CA0 + c2 : CA0 + c2 + 1] for c2 in range(2)]
            pM1 = {c2: mgroup(1, c2) for c2 in range(2)}
            for c2 in range(2):
                s1[c2] = work.tile([128, NH], F32, tag=f"ws1{c2}", name="ws1")
                nc.scalar.activation(
                    s1[c2][:], pM1[c2][:], mybir.ActivationFunctionType.Copy)
            pM2 = {c2: mgroup(2, c2) for c2 in range(2)}
            for c2 in range(2):
                u[c2] = work.tile([128, NH], F32, tag=f"wu{c2}", name="wu_")
                v_[c2] = work.tile([128, NH], F32, tag=f"wv{c2}", name="wv_")
                nc.vector.tensor_sub(u[c2][:], s1[c2][:], pM2[c2][:])
                nc.vector.tensor_add(v_[c2][:], s1[c2][:], pM2[c2][:])
            pM0 = {c2: mgroup(0, c2) for c2 in range(2)}
            for c2 in range(2):
                t0 = work.tile([128, NH], F32, tag=f"wt0{c2}", name="wt0")
                nc.vector.tensor_add(t0[:], v_[c2][:], pM0[c2][:])
                nc.scalar.activation(
                    y3[c2][:, 1:29:2, 1 : 1 + H],
                    t0[:].rearrange("p (i j) -> p i j", j=H),
                    mybir.ActivationFunctionType.Relu, bias=bias[c2], scale=1.0)
            pM3 = {c2: mgroup(3, c2) for c2 in range(2)}
            for c2 in range(2):
                t1 = work.tile([128, NH], F32, tag=f"wt1{c2}", name="wt1")
                nc.vector.tensor_sub(t1[:], u[c2][:], pM3[c2][:])
                nc.scalar.activation(
                    y3[c2][:, 2:29:2, 1 : 1 + H],
                    t1[:].rearrange("p (i j) -> p i j", j=H),
                    mybir.ActivationFunctionType.Relu, bias=bias[c2], scale=1.0)
            st["x3"] = x3
            st["y3"] = y3

        def emit_rng_stats(s):
            """Vector-side selection stats (exact f32 max/min)."""
            st = state[s]
            xf = st["xf"]
            mx = work.tile([128, 2], F32, tag="mx", name="mx", bufs=4)
            mn = work.tile([128, 2], F32, tag="mn", name="mn", bufs=4)
            rng_t = work.tile([128, 2], F32, tag="rng", name="rng", bufs=4)
            for c in range(2):
                nc.vector.tensor_reduce(
                    mx[:, c : c + 1], xf[c][:],
                    axis=mybir.AxisListType.X, op=mybir.AluOpType.max)
                nc.vector.tensor_reduce(
                    mn[:, c : c + 1], xf[c][:],
                    axis=mybir.AxisListType.X, op=mybir.AluOpType.min)
            nc.vector.tensor_sub(rng_t[:], mx[:], mn[:])
            st["rng"] = rng_t

        def emit_rng_selA(s):
            """PE transposes of the range columns (placed right after
            front(s)'s matmuls in the PE stream)."""
            st = state[s]
            rng_t = st["rng"]
            pbc = psbc.tile([128, 512], F32, tag="bc", name="bc")
            rngT = work.tile([128, 256], F32, tag="rngT", name="rngT", bufs=2)
            for h in range(2):
                nc.tensor.transpose(
                    pbc[0:1, 256 + 128 * h : 384 + 128 * h],
                    rng_t[:, h : h + 1], ident)
                nc.vector.tensor_copy(
                    rngT[0:1, 128 * h : 128 * (h + 1)],
                    pbc[0:1, 256 + 128 * h : 384 + 128 * h])
            st["pbc"], st["rngT"] = pbc, rngT

        def emit_rng_selB(s):
            """Broadcast matmul (exact: single 1.0*v products) + rank +
            one-hot permutation build."""
            st = state[s]
            pbc, rngT, rng_t = st.pop("pbc"), st.pop("rngT"), st.pop("rng")
            nc.tensor.matmul(
                pbc[:, 0:256], cs_t[0:1, ONE0 : ONE0 + 128],
                rngT[0:1, 0:256], start=True, stop=True)
            rank_t = work.tile([128, 2], F32, tag="rank", name="rank", bufs=4)
            P_t = work.tile([128, 256], adt, tag="pt", name="pt", bufs=4)
            for c in range(2):
                mk = work.tile([128, 256], F32, tag=f"mask{c}", name=f"mask{c}", bufs=4)
                nc.vector.tensor_scalar(
                    mk[:], pbc[:, 0:256], rng_t[:, c : c + 1], None,
                    mybir.AluOpType.is_lt)
                nc.vector.tensor_reduce(
                    rank_t[:, c : c + 1], mk[:],
                    axis=mybir.AxisListType.X, op=mybir.AluOpType.add)
                nc.vector.tensor_scalar(
                    P_t[:, c * 128 : (c + 1) * 128], iota, rank_t[:, c : c + 1],
                    None, mybir.AluOpType.is_equal)
            st["P_t"] = P_t

        def emit_mid_mm(s):
            """x_sel gather + border-value matmuls (PE work, depth-1 ahead)."""
            st = state[s]
            x3, P_t = st["x3"], st["P_t"]
            # gather selected channels via one-hot matmul
            x_sel = work.tile([128, H * H], adt, tag="xsel", name="xsel")
            for h in range(2):
                ps = psmm.tile([128, NH], F32, tag="mm", name="mm")
                for c in range(2):
                    nc.tensor.matmul(
                        ps[:], P_t[:, c * 128 : (c + 1) * 128],
                        x3[c][:, 1 + 14 * h : 15 + 14 * h, 1 : 1 + H],
                        start=(c == 0), stop=(c == 1))
                nc.scalar.activation(
                    x_sel[:, NH * h : NH * (h + 1)], ps[:],
                    mybir.ActivationFunctionType.Copy)
            s3 = x_sel[:].rearrange("p (i j) -> p i j", j=H)

            # border-value matmuls (identity-only contributions)
            pb = psbp.tile([128, NPB], F32, tag="pb", name="pb")
            for b in range(2):      # row border, classes (0,b), output row P=0
                nc.tensor.matmul(
                    pb[:, RB0 + b * 28 : RB0 + (b + 1) * 28],
                    wus(b), s3[:, 0, :], start=True, stop=True)
            for a in range(2):      # col border, classes (a,0), output col Q=0
                for h in range(2):
                    o = CB0 + (a * 2 + h) * 14
                    nc.tensor.matmul(
                        pb[:, o : o + 14], wus(a * 2),
                        s3[:, 14 * h : 14 * h + 14, 0],
                        start=True, stop=True)
            st["s3"], st["pb"] = s3, pb

        def emit_back(s):
            """Stage B/C: stride-2 convT + upsample branch, fused relu, store.
            h-outer so rows 0-27 can stream out while rows 28-55 compute."""
            st = state.pop(s)
            y3, s3, pb = st["y3"], st["s3"], st["pb"]
            G_t = work.tile([128, 56 * 56], F32, tag="g", name="g")
            g5 = G_t[:].rearrange("p (i x j y) -> p i x j y", x=2, y=2, j=H)
            rowtaps = {0: [(0, 1), (2, 0)], 1: [(1, 1)]}
            coltaps = rowtaps
            for h in range(2):
                for a in range(2):
                    for b in range(2):
                        k = a * 2 + b
                        n_taps = 2 * len(rowtaps[a]) * len(coltaps[b])
                        psC = psmm.tile([128, NH], F32, tag="mm", name="mm")
                        p3 = psC[:].rearrange("p (i j) -> p i j", j=H)
                        nc.tensor.matmul(
                            psC[:], wus(k), s3[:, 14 * h : 14 * h + 14, :],
                            start=True, stop=False)
                        n_mm = 0
                        for kh, ro in rowtaps[a]:
                            for kw, co_ in coltaps[b]:
                                for c in range(2):
                                    n_mm += 1
                                    nc.tensor.matmul(
                                        psC[:], w1s(c, kh * 3 + kw),
                                        y3[c][:, ro + 14 * h : ro + 14 * h + 14,
                                                 co_ : co_ + H],
                                        start=False, stop=(n_mm == n_taps))
                        i0 = 1 if (a == 0 and h == 0) else 0
                        j0 = 1 if b == 0 else 0
                        nc.scalar.activation(
                            g5[:, 14 * h + i0 : 14 * h + 14, a, j0:H, b],
                            p3[:, i0:14, j0:H],
                            mybir.ActivationFunctionType.Relu,
                            bias=cs_t[:, BI0 + k : BI0 + k + 1], scale=1.0)
                        # border fixes: output positions whose main-branch
                        # input is pure padding get relu(identity + border
                        # bias) instead
                        if a == 0 and h == 0:
                            nc.scalar.activation(
                                g5[:, 0, 0, :, b],
                                pb[:, RB0 + b * 28 : RB0 + b * 28 + 28],
                                mybir.ActivationFunctionType.Relu,
                                bias=cs_t[:, BB0 + k : BB0 + k + 1], scale=1.0)
                        if b == 0:
                            i0 = 1 if (a == 0 and h == 0) else 0
                            o = CB0 + (a * 2 + h) * 14
                            nc.scalar.activation(
                                g5[:, 14 * h + i0 : 14 * h + 14, a, 0, 0],
                                pb[:, o + i0 : o + 14],
                                mybir.ActivationFunctionType.Relu,
                                bias=cs_t[:, BB0 + k : BB0 + k + 1], scale=1.0)
                nc.sync.dma_start(
                    out_d[s][:, h * 1568 : (h + 1) * 1568],
                    G_t[:, h * 1568 : (h + 1) * 1568])

        # --- DMA emission order == arrival order: the head is exactly what
        # the first front group needs (xpb0 c0 + w2 c0), weights not needed
        # until the first mid/back phases go after all sample loads they
        # would otherwise delay. ---
        emit_xpb_load(0, c=0)
        nc.sync.dma_start(w2_t[0][:, 768 : 2 * 768], w2_d[0, 1])
        emit_xpb_load(0, c=1)
        nc.sync.dma_start(w2_t[1][:, 768 : 2 * 768], w2_d[1, 1])
        for xi in (2,):
            for c in range(2):
                nc.sync.dma_start(
                    w2_t[c][:, xi * 768 : (xi + 1) * 768], w2_d[c, xi])
        nc.sync.dma_start(cs_t[:], cs_d[:])
        for xi in (0, 3):
            for c in range(2):
                nc.sync.dma_start(
                    w2_t[c][:, xi * 768 : (xi + 1) * 768], w2_d[c, xi])
        emit_xpf_load(0)
        emit_xpb_load(1)
        emit_xpf_load(1)
        nc.sync.dma_start(wu_t[:], wu_d[:])
        emit_xpb_load(2)
        emit_xpb_load(3)
        emit_xpf_load(2)
        emit_xpf_load(3)
        for c in range(2):
            nc.sync.dma_start(w1_t[c][:], w1_d[c])

        # software pipeline (PE stream): F0, F1, tr0, bc0, M0 | tr1, bc1,
        # F2, M1, B0 | tr2, bc2, F3, M2, B1 | tr3, bc3, M3, B2 | B3. The
        # selection chain for sample s+1 leads each iteration: its broadcast
        # rides between front phases and its rank chain (Vector) finishes
        # while the PE is inside front(s+2), so mid(s+1) is never gated.
        emit_front(0)
        emit_rng_stats(0)
        emit_front(1)
        emit_rng_selA(0)
        emit_rng_selB(0)
        emit_rng_stats(1)
        emit_mid_mm(0)
        for s in range(S):
            if s + 1 <= S - 2:
                emit_rng_selA(s + 1)
                emit_rng_selB(s + 1)
            if s + 2 < S:
                # sample 2 fills the pipeline while Vector is still busy with
                # sample 1's combines: route its c1 transforms to GpSimd,
                # which is idle by then.
                emit_front(s + 2, c1_eng=nc.gpsimd if s + 2 == 2 else None)
                emit_rng_stats(s + 2)
            if s + 1 < S:
                emit_mid_mm(s + 1)
            if s == S - 3:
                # last sample's selection chain rides inside this iteration so
                # its rank chain is hidden under back(s) instead of exposed at
                # the pipeline tail.
                emit_rng_selA(S - 1)
                emit_rng_selB(S - 1)
            emit_back(s)
    _split_multi_waits(nc)
    return nc


def prep_inputs(x, w_conv2, w_conv1, w_up, bias1a, bias1b, bias2a, bias2b, scale,
                w_np_dt=np.float32):
    """Host-side input transforms shared by all cores (weights/constants)."""
    f = np.float32
    b1a, b1b = f(bias1a[0]), f(bias1b[0])
    b2a, b2b, sc = f(bias2a[0]), f(bias2b[0]), f(scale[0])

    xp = np.zeros((B, 2, 128, HP, HP), dtype=np.float32)
    xp[:, :, :, 1 : 1 + H, 1 : 1 + H] = x.reshape(B, 2, 128, H, H)
    xpb = xp.reshape(B, 2, 128, HP * HP).astype(w_np_dt)
    xpf = np.ascontiguousarray(x.reshape(B, 2, 128, H * H), dtype=np.float32)

    wf = w_conv2[:, :, ::-1, ::-1]  # flip -> correlation form
    # 1-D Winograd F(2,3) weight transform along kh (correlation taps g0..g2):
    # U0 = g0, U1 = (g0+g1+g2)/2, U2 = (g0-g1+g2)/2, U3 = g2.
    U = np.stack([
        wf[:, :, 0, :],
        0.5 * (wf[:, :, 0, :] + wf[:, :, 1, :] + wf[:, :, 2, :]),
        0.5 * (wf[:, :, 0, :] - wf[:, :, 1, :] + wf[:, :, 2, :]),
        wf[:, :, 2, :],
    ])                                                  # [4, 256ci, 256co, 3kw]
    w2 = np.ascontiguousarray(
        U.reshape(4, 2, 128, 2, 128, 3).transpose(1, 0, 2, 5, 3, 4)
    ).reshape(2, 4, 128, 3 * 2 * 128)
    w1s = (w_conv1 * sc).astype(np.float32)
    w1 = np.ascontiguousarray(
        w1s.reshape(2, 128, 128, 3, 3).transpose(0, 1, 3, 4, 2)
    ).reshape(2, 128, 9 * 128)
    wu = np.ascontiguousarray(w_up.transpose(0, 2, 3, 1)).reshape(128, 4 * 128)

    cs = np.zeros((128, NCONST), dtype=np.float32)
    cs[:, IDN0 : IDN0 + 128] = np.eye(128, dtype=np.float32)
    cs[:, IOT0 : IOT0 + 128] = np.arange(128, 256, dtype=np.float32)[None, :]
    cA = b1a * w_conv2.sum(axis=(0, 2, 3)) + b1b               # [256]
    cs[:, CA0 : CA0 + 2] = cA.reshape(2, 128).T
    cI = b1a * w_up.sum(axis=0)                                # [128,2,2]
    # fold bias2a into the per-class interior bias: conv(y + b2a) adds
    # b2a * (sum of the class's taps over all input channels) per output
    # channel (exact for interior positions; b2a is 0 under Fixup init).
    wsum = w1s.sum(axis=0)                                     # [128,3,3]
    ktaps = {0: [0, 2], 1: [1]}
    for a in range(2):
        for b_ in range(2):
            Sk = wsum[:, ktaps[a], :][:, :, ktaps[b_]].sum(axis=(1, 2))
            cs[:, BI0 + a * 2 + b_] = cI[:, a, b_] + b2b + b2a * Sk
            cs[:, BB0 + a * 2 + b_] = cI[:, a, b_]
    cs[:, ONE0 : ONE0 + 128] = 1.0

    shared = {"w2": w2.astype(w_np_dt), "w1": w1.astype(w_np_dt),
              "wu": wu.astype(w_np_dt), "cs": cs}
    in_maps = []
    for i in range(N_CORES):
        m = dict(shared)
        m["xpb"] = np.ascontiguousarray(xpb[i * S : (i + 1) * S])
        m["xpf"] = np.ascontiguousarray(xpf[i * S : (i + 1) * S])
        in_maps.append(m)
    return in_maps


_NC_CACHE = {}


def _get_nc(mm_dt):
    key = str(mm_dt)
    if key not in _NC_CACHE:
        _NC_CACHE[key] = build_nc(mm_dt)
    return _NC_CACHE[key]


MM_DT = mybir.dt.bfloat16


def run(inputs, mm_dt=None, trace=False):
    mm_dt = MM_DT if mm_dt is None else mm_dt
    nc = _get_nc(mm_dt)
    in_maps = prep_inputs(**inputs, w_np_dt=mybir.dt.np(mm_dt))
    res = run_bass_kernel_spmd(nc, in_maps, list(range(N_CORES)), trace=trace)
    out = np.empty((B, CO1, 56, 56), dtype=np.float32)
    for i in range(N_CORES):
        out[i * S : (i + 1) * S] = res.results[i]["out"].reshape(S, CO1, 56, 56)
    return out, res


def kernel(**inputs):
    out, _ = run(inputs)
    return out
